# revision 14
# baseline (speedup 1.0000x reference)
"""Ragged masked softmax + dropout kernel for 8 Trainium2 NeuronCores.

Computation (per softmax row of length s_i, batch i, any head):
    x' = x + mask_i            (mask_i broadcasts over all heads*s_i rows)
    out = exp(x' - max(x')) / sum(exp(x' - max(x'))) * keep / (1 - p)
where keep = jax.random.bernoulli(key(42), 1-p, (total,)) -- reproduced
bit-exactly on host (jax CPU threefry) and shipped as uint8.

Sharding: each batch i has heads*s_i rows of length s_i, contiguous in the
flat buffer.  Core d owns rows [d*rpc_i, (d+1)*rpc_i) of EVERY batch
(rpc_i = heads*s_i/8), i.e. one contiguous flat span per batch per core.
All cores get identical-size, identical-structure shards -> one SPMD program.

Device layout: per-core shards are host-permuted per segment from
[T*128 rows, s] row-major to partition-major [128][T][s] so every DMA moves
long contiguous per-partition lines (k*s*4B for x/y groups, T*s B for the
whole keep segment) instead of 1-4KB row lines.  Loads issue on the sync
HWDGE ring, stores on the scalar (ACT) ring.
"""

import os
import sys

import numpy as np

os.environ.setdefault("MYCRO_LOCAL_CACHE", "1")

for _p in ("/opt/trn_rl_repo", "/root/.axon_site/_ro/trn_rl_repo"):
    if os.path.isdir(_p) and _p not in sys.path:
        sys.path.append(_p)

_N_CORES = 8
_DROPOUT_P = 0.1

_PROGRAM_CACHE = {}
# exposed for test harness: BassKernelResults of the most recent run
LAST_RESULTS = None


def _groups(T, s):
    """Split T tiles into groups with ~16KB f32 per-partition DMA lines."""
    k = max(1, min(T, 16384 // (4 * s)))
    out = []
    t = 0
    while t < T:
        g = min(k, T - t)
        out.append((t, g))
        t += g
    return out


def _build_bass(seg_shapes, mask_offsets, mask_total, per_core_total,
                use_max=False):
    """seg_shapes: list of (rows_per_core, seqlen); mask_offsets: per-segment
    offsets into the flat mask tensor.  x/keep/y DRAM tensors hold the
    host-permuted partition-major layout described in the module docstring.

    use_max=False skips the rowmax subtraction (safe for bounded inputs:
    exp(x+m) cannot overflow and masked entries underflow to exactly 0);
    use_max=True is the numerically-safe general path."""
    import concourse.bass as bass
    import concourse.bacc as bacc
    import concourse.tile as tile
    from concourse import mybir
    from contextlib import ExitStack

    f32 = mybir.dt.float32
    u8 = mybir.dt.uint8
    EXP = mybir.ActivationFunctionType.Exp
    COPY = mybir.ActivationFunctionType.Copy
    ADD = mybir.AluOpType.add
    MAX = mybir.AluOpType.max
    MULT = mybir.AluOpType.mult
    inv_keep = float(1.0 / (1.0 - _DROPOUT_P))

    nc = bacc.Bacc("TRN2", target_bir_lowering=False)
    x = nc.dram_tensor("x", [per_core_total], f32, kind="ExternalInput")
    keep = nc.dram_tensor("keep", [per_core_total], u8, kind="ExternalInput")
    mask = nc.dram_tensor("mask", [mask_total], f32, kind="ExternalInput")
    y = nc.dram_tensor("y", [per_core_total], f32, kind="ExternalOutput")

    with tile.TileContext(nc) as tc, ExitStack() as ctx:
        xp = ctx.enter_context(tc.tile_pool(name="xp", bufs=4))
        kp = ctx.enter_context(tc.tile_pool(name="kp", bufs=4))
        xmp = ctx.enter_context(tc.tile_pool(name="xmp", bufs=3))
        ep = ctx.enter_context(tc.tile_pool(name="ep", bufs=3))
        kfp = ctx.enter_context(tc.tile_pool(name="kfp", bufs=3))
        op = ctx.enter_context(tc.tile_pool(name="op", bufs=2))
        mp = ctx.enter_context(tc.tile_pool(name="mp", bufs=2))
        st = ctx.enter_context(tc.tile_pool(name="st", bufs=6))

        c = 0
        for (rpc, s), moff in zip(seg_shapes, mask_offsets):
            T = (rpc + 127) // 128
            assert rpc == T * 128, "permuted layout needs rpc % 128 == 0"

            mt = mp.tile([128, s], f32, tag="mt")
            msrc = mask[moff : moff + s]
            bsrc = bass.AP(
                tensor=msrc.tensor,
                offset=msrc.offset,
                ap=[[0, 128]] + [list(a) for a in msrc.ap],
            )
            nc.gpsimd.dma_start(out=mt[:, :], in_=bsrc)

            # permuted views: [128 partitions, T, s]
            xv = x[c : c + rpc * s].rearrange("(p t s) -> p t s", p=128, s=s)
            kv = keep[c : c + rpc * s].rearrange("(p t s) -> p t s", p=128, s=s)
            yv = y[c : c + rpc * s].rearrange("(p t s) -> p t s", p=128, s=s)

            for t0, g in _groups(T, s):
                xg = xp.tile([128, g, s], f32, tag="xg")
                nc.sync.dma_start(out=xg[:, :, :], in_=xv[:, t0 : t0 + g, :])
                kt = kp.tile([128, g, s], u8, tag="kt")
                nc.sync.dma_start(out=kt[:, :, :], in_=kv[:, t0 : t0 + g, :])
                og = op.tile([128, g, s], f32, tag="og")

                for j in range(g):
                    # xm = x + mask
                    xm = xmp.tile([128, s], f32, tag="xm")
                    nc.vector.tensor_tensor(xm[:], xg[:, j, :], mt[:], ADD)
                    if use_max:
                        mn = st.tile([128, 1], f32, tag="mn")
                        nc.vector.tensor_reduce(
                            mn[:],
                            xm[:],
                            axis=mybir.AxisListType.X,
                            op=MAX,
                            negate=True,
                        )
                    # e = exp(xm [- rowmax]); sm = rowsum(e)
                    et = ep.tile([128, s], f32, tag="et")
                    sm = st.tile([128, 1], f32, tag="sm")
                    nc.scalar.activation(
                        out=et[:],
                        in_=xm[:],
                        func=EXP,
                        bias=mn[:] if use_max else 0.0,
                        scale=1.0,
                        accum_out=sm[:],
                    )
                    rc = st.tile([128, 1], f32, tag="rc")
                    nc.vector.reciprocal(out=rc[:], in_=sm[:])
                    rcs = st.tile([128, 1], f32, tag="rcs")
                    nc.vector.tensor_scalar_mul(rcs[:], rc[:], inv_keep)
                    # keepf = keep * (1/(0.9*s)) via ACT copy w/ scale AP
                    kf = kfp.tile([128, s], f32, tag="kf")
                    nc.scalar.activation(
                        out=kf[:],
                        in_=kt[:, j, :],
                        func=COPY,
                        scale=rcs[:],
                    )
                    nc.vector.tensor_tensor(og[:, j, :], et[:], kf[:], MULT)
                # store the whole group on the ACT HWDGE ring
                nc.scalar.dma_start(out=yv[:, t0 : t0 + g, :], in_=og[:, :, :])
            c += rpc * s
        assert c == per_core_total
    nc.finalize()
    return nc


def _keep_mask_u8(total):
    """Reproduce the reference dropout keep mask bit-exactly (jax threefry)."""
    import jax

    try:
        dev = jax.devices("cpu")[0]
    except RuntimeError:
        dev = None
    if dev is not None:
        with jax.default_device(dev):
            k = jax.random.bernoulli(
                jax.random.key(42), 1.0 - _DROPOUT_P, (total,)
            )
            k = np.asarray(k)
    else:
        k = np.asarray(
            jax.random.bernoulli(jax.random.key(42), 1.0 - _DROPOUT_P, (total,))
        )
    return k.astype(np.uint8)


def kernel(input, mask, seqlen, batch, heads):
    from concourse.bass_utils import run_bass_kernel_spmd

    global LAST_RESULTS

    x = np.ascontiguousarray(np.asarray(input, dtype=np.float32).reshape(-1))
    m = np.ascontiguousarray(np.asarray(mask, dtype=np.float32).reshape(-1))
    seqs = [int(v) for v in np.asarray(seqlen).reshape(-1)]
    b = int(batch)
    h = int(heads)
    seqs = seqs[:b]
    assert all(h * s % _N_CORES == 0 for s in seqs)
    assert all((h * s // _N_CORES) % 128 == 0 for s in seqs)

    total = int(sum(h * s * s for s in seqs))
    mask_total = int(sum(seqs))
    assert x.size == total and m.size == mask_total

    seg_shapes = [((h * s) // _N_CORES, s) for s in seqs]
    per_core_total = sum(r * s for r, s in seg_shapes)
    moffs = np.concatenate([[0], np.cumsum(seqs)])[:-1].astype(np.int64)
    bases = np.concatenate(
        [[0], np.cumsum([h * s * s for s in seqs])]
    )[:-1].astype(np.int64)

    # The fast path skips rowmax subtraction.  That is exact enough when
    # exp(x) cannot overflow f32 and no softmax row is entirely masked
    # (an all-masked row needs the max-subtracted degenerate 1/s output).
    finite_mask = m > -1e8
    all_masked_row = any(
        not finite_mask[mo : mo + s].any() for mo, s in zip(moffs, seqs)
    )
    use_max = bool(all_masked_row or (x.size and np.abs(x).max() > 60.0))

    key = (tuple(seqs), h, use_max)
    if key not in _PROGRAM_CACHE:
        _PROGRAM_CACHE[key] = _build_bass(
            seg_shapes, list(moffs), mask_total, per_core_total, use_max=use_max
        )
    nc = _PROGRAM_CACHE[key]

    keep = _keep_mask_u8(total)

    # shard + permute: core d gets, for each batch i, the contiguous flat
    # span [base_i + d*n, base_i + (d+1)*n) (n = rpc_i*s_i), reordered from
    # [T,128,s] row-major tiles to partition-major [128,T,s].
    in_maps = []
    for d in range(_N_CORES):
        xs = np.empty(per_core_total, dtype=np.float32)
        ks = np.empty(per_core_total, dtype=np.uint8)
        c = 0
        for i, (r, s) in enumerate(seg_shapes):
            n = r * s
            T = r // 128
            src = slice(bases[i] + d * n, bases[i] + (d + 1) * n)
            xs[c : c + n] = (
                x[src].reshape(T, 128, s).transpose(1, 0, 2).reshape(-1)
            )
            ks[c : c + n] = (
                keep[src].reshape(T, 128, s).transpose(1, 0, 2).reshape(-1)
            )
            c += n
        in_maps.append({"x": xs, "keep": ks, "mask": m})

    trace = bool(os.environ.get("KERNEL_TRACE"))
    if not trace:
        # an ambient BASS_TRACE would route through the axon NTFF hook,
        # which this image only has when the test harness installs it
        os.environ["BASS_NEVER_TRACE"] = "1"
    res = run_bass_kernel_spmd(
        nc, in_maps, core_ids=list(range(_N_CORES)), trace=trace
    )
    LAST_RESULTS = res

    out = np.empty(total, dtype=np.float32)
    for d in range(_N_CORES):
        yd = res.results[d]["y"]
        c = 0
        for i, (r, s) in enumerate(seg_shapes):
            n = r * s
            T = r // 128
            out[bases[i] + d * n : bases[i] + (d + 1) * n] = (
                yd[c : c + n].reshape(128, T, s).transpose(1, 0, 2).reshape(-1)
            )
            c += n
    return out


# revision 17
# speedup vs baseline: 1.0449x; 1.0449x over previous
"""Ragged masked softmax + dropout kernel for 8 Trainium2 NeuronCores.

Computation (per softmax row of length s_i, batch i, any head):
    x' = x + mask_i            (mask_i broadcasts over all heads*s_i rows)
    out = exp(x' - max(x')) / sum(exp(x' - max(x'))) * keep / (1 - p)
where keep = jax.random.bernoulli(key(42), 1-p, (total,)) -- reproduced
bit-exactly on host (jax CPU threefry) and shipped as uint8.

Sharding: each batch i has heads*s_i rows of length s_i, contiguous in the
flat buffer.  Core d owns rows [d*rpc_i, (d+1)*rpc_i) of EVERY batch
(rpc_i = heads*s_i/8), i.e. one contiguous flat span per batch per core.
All cores get identical-size, identical-structure shards -> one SPMD program.

Device layout: per-core shards are host-permuted per segment from
[T*128 rows, s] row-major to partition-major [128][T][s] so every DMA moves
long contiguous per-partition lines (k*s*4B for x/y groups, T*s B for the
whole keep segment) instead of 1-4KB row lines.  Loads issue on the sync
HWDGE ring, stores on the scalar (ACT) ring.
"""

import os
import sys

import numpy as np

os.environ.setdefault("MYCRO_LOCAL_CACHE", "1")

for _p in ("/opt/trn_rl_repo", "/root/.axon_site/_ro/trn_rl_repo"):
    if os.path.isdir(_p) and _p not in sys.path:
        sys.path.append(_p)

_N_CORES = 8
_DROPOUT_P = 0.1

_PROGRAM_CACHE = {}
# exposed for test harness: BassKernelResults of the most recent run
LAST_RESULTS = None


def _groups(T, s):
    """Split T tiles into groups with ~16KB f32 per-partition DMA lines."""
    k = max(1, min(T, 16384 // (4 * s)))
    out = []
    t = 0
    while t < T:
        g = min(k, T - t)
        out.append((t, g))
        t += g
    return out


def _build_bass(seg_shapes, mask_offsets, mask_total, per_core_total,
                use_max=False):
    """seg_shapes: list of (rows_per_core, seqlen); mask_offsets: per-segment
    offsets into the flat mask tensor.  x/keep/y DRAM tensors hold the
    host-permuted partition-major layout described in the module docstring.

    use_max=False skips the rowmax subtraction (safe for bounded inputs:
    exp(x+m) cannot overflow and masked entries underflow to exactly 0);
    use_max=True is the numerically-safe general path."""
    import concourse.bass as bass
    import concourse.bacc as bacc
    import concourse.tile as tile
    from concourse import mybir
    from contextlib import ExitStack

    f32 = mybir.dt.float32
    u8 = mybir.dt.uint8
    EXP = mybir.ActivationFunctionType.Exp
    COPY = mybir.ActivationFunctionType.Copy
    ADD = mybir.AluOpType.add
    MAX = mybir.AluOpType.max
    MULT = mybir.AluOpType.mult
    inv_keep = float(1.0 / (1.0 - _DROPOUT_P))

    nc = bacc.Bacc("TRN2", target_bir_lowering=False)
    x = nc.dram_tensor("x", [per_core_total], f32, kind="ExternalInput")
    keep = nc.dram_tensor("keep", [per_core_total], u8, kind="ExternalInput")
    mask = nc.dram_tensor("mask", [mask_total], f32, kind="ExternalInput")
    y = nc.dram_tensor("y", [per_core_total], f32, kind="ExternalOutput")

    with tile.TileContext(nc) as tc, ExitStack() as ctx:
        xp = ctx.enter_context(tc.tile_pool(name="xp", bufs=3))
        kp = ctx.enter_context(tc.tile_pool(name="kp", bufs=2))
        xmp = ctx.enter_context(tc.tile_pool(name="xmp", bufs=3))
        ep = ctx.enter_context(tc.tile_pool(name="ep", bufs=3))
        kfp = ctx.enter_context(tc.tile_pool(name="kfp", bufs=3))
        op = ctx.enter_context(tc.tile_pool(name="op", bufs=2))
        mp = ctx.enter_context(tc.tile_pool(name="mp", bufs=2))
        st = ctx.enter_context(tc.tile_pool(name="st", bufs=8))

        c = 0
        for (rpc, s), moff in zip(seg_shapes, mask_offsets):
            T = (rpc + 127) // 128
            assert rpc == T * 128, "permuted layout needs rpc % 128 == 0"

            mt = mp.tile([128, s], f32, tag="mt")
            msrc = mask[moff : moff + s]
            bsrc = bass.AP(
                tensor=msrc.tensor,
                offset=msrc.offset,
                ap=[[0, 128]] + [list(a) for a in msrc.ap],
            )
            nc.gpsimd.dma_start(out=mt[:, :], in_=bsrc)

            # permuted views: [128 partitions, T, s]
            xv = x[c : c + rpc * s].rearrange("(p t s) -> p t s", p=128, s=s)
            kv = keep[c : c + rpc * s].rearrange("(p t s) -> p t s", p=128, s=s)
            yv = y[c : c + rpc * s].rearrange("(p t s) -> p t s", p=128, s=s)

            # whole keep segment in one DMA (contiguous T*s per partition)
            kt = kp.tile([128, T, s], u8, tag="kt")
            nc.sync.dma_start(out=kt[:, :, :], in_=kv)

            for t0, g in _groups(T, s):
                xg = xp.tile([128, g, s], f32, tag="xg")
                nc.sync.dma_start(out=xg[:, :, :], in_=xv[:, t0 : t0 + g, :])
                og = op.tile([128, g, s], f32, tag="og")

                for j in range(g):
                    # xm = x + mask
                    xm = xmp.tile([128, s], f32, tag="xm")
                    nc.vector.tensor_tensor(xm[:], xg[:, j, :], mt[:], ADD)
                    if use_max:
                        mn = st.tile([128, 1], f32, tag="mn")
                        nc.vector.tensor_reduce(
                            mn[:],
                            xm[:],
                            axis=mybir.AxisListType.X,
                            op=MAX,
                            negate=True,
                        )
                    # e = exp(xm [- rowmax]); sm = rowsum(e)
                    et = ep.tile([128, s], f32, tag="et")
                    sm = st.tile([128, 1], f32, tag="sm")
                    nc.scalar.activation(
                        out=et[:],
                        in_=xm[:],
                        func=EXP,
                        bias=mn[:] if use_max else 0.0,
                        scale=1.0,
                        accum_out=sm[:],
                    )
                    rc = st.tile([128, 1], f32, tag="rc")
                    nc.vector.reciprocal(out=rc[:], in_=sm[:])
                    rcs = st.tile([128, 1], f32, tag="rcs")
                    nc.vector.tensor_scalar_mul(rcs[:], rc[:], inv_keep)
                    # keepf = keep * (1/(0.9*s)) via ACT copy w/ scale AP
                    kf = kfp.tile([128, s], f32, tag="kf")
                    nc.scalar.activation(
                        out=kf[:],
                        in_=kt[:, t0 + j, :],
                        func=COPY,
                        scale=rcs[:],
                    )
                    nc.vector.tensor_tensor(og[:, j, :], et[:], kf[:], MULT)
                # store the whole group on the ACT HWDGE ring
                nc.scalar.dma_start(out=yv[:, t0 : t0 + g, :], in_=og[:, :, :])
            c += rpc * s
        assert c == per_core_total
    nc.finalize()
    return nc


def _keep_mask_u8(total):
    """Reproduce the reference dropout keep mask bit-exactly (jax threefry)."""
    import jax

    try:
        dev = jax.devices("cpu")[0]
    except RuntimeError:
        dev = None
    if dev is not None:
        with jax.default_device(dev):
            k = jax.random.bernoulli(
                jax.random.key(42), 1.0 - _DROPOUT_P, (total,)
            )
            k = np.asarray(k)
    else:
        k = np.asarray(
            jax.random.bernoulli(jax.random.key(42), 1.0 - _DROPOUT_P, (total,))
        )
    return k.astype(np.uint8)


def kernel(input, mask, seqlen, batch, heads):
    from concourse.bass_utils import run_bass_kernel_spmd

    global LAST_RESULTS

    x = np.ascontiguousarray(np.asarray(input, dtype=np.float32).reshape(-1))
    m = np.ascontiguousarray(np.asarray(mask, dtype=np.float32).reshape(-1))
    seqs = [int(v) for v in np.asarray(seqlen).reshape(-1)]
    b = int(batch)
    h = int(heads)
    seqs = seqs[:b]
    assert all(h * s % _N_CORES == 0 for s in seqs)
    assert all((h * s // _N_CORES) % 128 == 0 for s in seqs)

    total = int(sum(h * s * s for s in seqs))
    mask_total = int(sum(seqs))
    assert x.size == total and m.size == mask_total

    seg_shapes = [((h * s) // _N_CORES, s) for s in seqs]
    per_core_total = sum(r * s for r, s in seg_shapes)
    moffs = np.concatenate([[0], np.cumsum(seqs)])[:-1].astype(np.int64)
    bases = np.concatenate(
        [[0], np.cumsum([h * s * s for s in seqs])]
    )[:-1].astype(np.int64)

    # The fast path skips rowmax subtraction.  That is exact enough when
    # exp(x) cannot overflow f32 and no softmax row is entirely masked
    # (an all-masked row needs the max-subtracted degenerate 1/s output).
    finite_mask = m > -1e8
    all_masked_row = any(
        not finite_mask[mo : mo + s].any() for mo, s in zip(moffs, seqs)
    )
    use_max = bool(all_masked_row or (x.size and np.abs(x).max() > 60.0))

    key = (tuple(seqs), h, use_max)
    if key not in _PROGRAM_CACHE:
        _PROGRAM_CACHE[key] = _build_bass(
            seg_shapes, list(moffs), mask_total, per_core_total, use_max=use_max
        )
    nc = _PROGRAM_CACHE[key]

    keep = _keep_mask_u8(total)

    # shard + permute: core d gets, for each batch i, the contiguous flat
    # span [base_i + d*n, base_i + (d+1)*n) (n = rpc_i*s_i), reordered from
    # [T,128,s] row-major tiles to partition-major [128,T,s].
    in_maps = []
    for d in range(_N_CORES):
        xs = np.empty(per_core_total, dtype=np.float32)
        ks = np.empty(per_core_total, dtype=np.uint8)
        c = 0
        for i, (r, s) in enumerate(seg_shapes):
            n = r * s
            T = r // 128
            src = slice(bases[i] + d * n, bases[i] + (d + 1) * n)
            xs[c : c + n] = (
                x[src].reshape(T, 128, s).transpose(1, 0, 2).reshape(-1)
            )
            ks[c : c + n] = (
                keep[src].reshape(T, 128, s).transpose(1, 0, 2).reshape(-1)
            )
            c += n
        in_maps.append({"x": xs, "keep": ks, "mask": m})

    trace = bool(os.environ.get("KERNEL_TRACE"))
    if not trace:
        # an ambient BASS_TRACE would route through the axon NTFF hook,
        # which this image only has when the test harness installs it
        os.environ["BASS_NEVER_TRACE"] = "1"
    res = run_bass_kernel_spmd(
        nc, in_maps, core_ids=list(range(_N_CORES)), trace=trace
    )
    LAST_RESULTS = res

    out = np.empty(total, dtype=np.float32)
    for d in range(_N_CORES):
        yd = res.results[d]["y"]
        c = 0
        for i, (r, s) in enumerate(seg_shapes):
            n = r * s
            T = r // 128
            out[bases[i] + d * n : bases[i] + (d + 1) * n] = (
                yd[c : c + n].reshape(128, T, s).transpose(1, 0, 2).reshape(-1)
            )
            c += n
    return out


# revision 21
# speedup vs baseline: 1.0599x; 1.0143x over previous
"""Ragged masked softmax + dropout kernel for 8 Trainium2 NeuronCores.

Computation (per softmax row of length s_i, batch i, any head):
    x' = x + mask_i            (mask_i broadcasts over all heads*s_i rows)
    out = exp(x' - max(x')) / sum(exp(x' - max(x'))) * keep / (1 - p)
where keep = jax.random.bernoulli(key(42), 1-p, (total,)) -- reproduced
bit-exactly on host (jax CPU threefry) and shipped as uint8.

Sharding: each batch i has heads*s_i rows of length s_i, contiguous in the
flat buffer.  Core d owns rows [d*rpc_i, (d+1)*rpc_i) of EVERY batch
(rpc_i = heads*s_i/8), i.e. one contiguous flat span per batch per core.
All cores get identical-size, identical-structure shards -> one SPMD program.

Device layout: per-core shards are host-permuted per segment from
[T*128 rows, s] row-major to partition-major [128][T][s] so every DMA moves
long contiguous per-partition lines (k*s*4B for x/y groups, T*s B for the
whole keep segment) instead of 1-4KB row lines.  Loads issue on the sync
HWDGE ring, stores on the scalar (ACT) ring.
"""

import os
import sys

import numpy as np

os.environ.setdefault("MYCRO_LOCAL_CACHE", "1")

for _p in ("/opt/trn_rl_repo", "/root/.axon_site/_ro/trn_rl_repo"):
    if os.path.isdir(_p) and _p not in sys.path:
        sys.path.append(_p)

_N_CORES = 8
_DROPOUT_P = 0.1

_PROGRAM_CACHE = {}
# exposed for test harness: BassKernelResults of the most recent run
LAST_RESULTS = None


def _groups(T, s, split_first=False):
    """Split T tiles into groups with ~16KB f32 per-partition DMA lines.
    split_first carves a 1-tile group off the front so the first compute
    can start as soon as one tile's worth of data lands."""
    k = max(1, min(T, 16384 // (4 * s)))
    out = []
    t = 0
    if split_first and k > 1:
        out.append((0, 1))
        t = 1
    while t < T:
        g = min(k, T - t)
        out.append((t, g))
        t += g
    return out


def _build_bass(seg_shapes, mask_offsets, mask_total, per_core_total,
                use_max=False):
    """seg_shapes: list of (rows_per_core, seqlen); mask_offsets: per-segment
    offsets into the flat mask tensor.  x/keep/y DRAM tensors hold the
    host-permuted partition-major layout described in the module docstring.

    use_max=False skips the rowmax subtraction (safe for bounded inputs:
    exp(x+m) cannot overflow and masked entries underflow to exactly 0);
    use_max=True is the numerically-safe general path."""
    import concourse.bass as bass
    import concourse.bacc as bacc
    import concourse.tile as tile
    from concourse import mybir
    from contextlib import ExitStack

    f32 = mybir.dt.float32
    u8 = mybir.dt.uint8
    EXP = mybir.ActivationFunctionType.Exp
    COPY = mybir.ActivationFunctionType.Copy
    ADD = mybir.AluOpType.add
    MAX = mybir.AluOpType.max
    MULT = mybir.AluOpType.mult
    inv_keep = float(1.0 / (1.0 - _DROPOUT_P))

    nc = bacc.Bacc("TRN2", target_bir_lowering=False)
    x = nc.dram_tensor("x", [per_core_total], f32, kind="ExternalInput")
    keep = nc.dram_tensor("keep", [per_core_total], u8, kind="ExternalInput")
    mask = nc.dram_tensor("mask", [mask_total], f32, kind="ExternalInput")
    y = nc.dram_tensor("y", [per_core_total], f32, kind="ExternalOutput")

    with tile.TileContext(nc) as tc, ExitStack() as ctx:
        xp = ctx.enter_context(tc.tile_pool(name="xp", bufs=3))
        kp = ctx.enter_context(tc.tile_pool(name="kp", bufs=2))
        xmp = ctx.enter_context(tc.tile_pool(name="xmp", bufs=3))
        ep = ctx.enter_context(tc.tile_pool(name="ep", bufs=3))
        op = ctx.enter_context(tc.tile_pool(name="op", bufs=2))
        mp = ctx.enter_context(tc.tile_pool(name="mp", bufs=2))
        st = ctx.enter_context(tc.tile_pool(name="st", bufs=8))

        c = 0
        for (rpc, s), moff in zip(seg_shapes, mask_offsets):
            T = (rpc + 127) // 128
            assert rpc == T * 128, "permuted layout needs rpc % 128 == 0"

            mt = mp.tile([128, s], f32, tag="mt")
            msrc = mask[moff : moff + s]
            bsrc = bass.AP(
                tensor=msrc.tensor,
                offset=msrc.offset,
                ap=[[0, 128]] + [list(a) for a in msrc.ap],
            )
            nc.gpsimd.dma_start(out=mt[:, :], in_=bsrc)

            # permuted views: [128 partitions, T, s]
            xv = x[c : c + rpc * s].rearrange("(p t s) -> p t s", p=128, s=s)
            kv = keep[c : c + rpc * s].rearrange("(p t s) -> p t s", p=128, s=s)
            yv = y[c : c + rpc * s].rearrange("(p t s) -> p t s", p=128, s=s)

            # whole keep segment in one DMA (contiguous T*s per partition);
            # issued after the first x group so first compute isn't stuck
            # behind it in the sync-ring FIFO
            kt = kp.tile([128, T, s], u8, tag="kt")
            kt_issued = False

            for t0, g in _groups(T, s, split_first=(c == 0)):
                xg = xp.tile([128, g, s], f32, tag="xg")
                nc.sync.dma_start(out=xg[:, :, :], in_=xv[:, t0 : t0 + g, :])
                if not kt_issued:
                    nc.sync.dma_start(out=kt[:, :, :], in_=kv)
                    kt_issued = True
                og = op.tile([128, g, s], f32, tag="og")

                for j in range(g):
                    # xm = x + mask
                    xm = xmp.tile([128, s], f32, tag="xm")
                    nc.vector.tensor_tensor(xm[:], xg[:, j, :], mt[:], ADD)
                    if use_max:
                        mn = st.tile([128, 1], f32, tag="mn")
                        nc.vector.tensor_reduce(
                            mn[:],
                            xm[:],
                            axis=mybir.AxisListType.X,
                            op=MAX,
                            negate=True,
                        )
                    # e = exp(xm [- rowmax]); sm = rowsum(e)
                    et = ep.tile([128, s], f32, tag="et")
                    sm = st.tile([128, 1], f32, tag="sm")
                    nc.scalar.activation(
                        out=et[:],
                        in_=xm[:],
                        func=EXP,
                        bias=mn[:] if use_max else 0.0,
                        scale=1.0,
                        accum_out=sm[:],
                    )
                    rc = st.tile([128, 1], f32, tag="rc")
                    nc.vector.reciprocal(out=rc[:], in_=sm[:])
                    rcs = st.tile([128, 1], f32, tag="rcs")
                    nc.vector.tensor_scalar_mul(rcs[:], rc[:], inv_keep)
                    # out = (e * 1/(0.9*s)) * keep in one fused DVE op
                    # (u8 keep is converted on the fly)
                    nc.vector.scalar_tensor_tensor(
                        og[:, j, :],
                        et[:],
                        rcs[:],
                        kt[:, t0 + j, :],
                        op0=MULT,
                        op1=MULT,
                    )
                # store the whole group on the ACT HWDGE ring
                nc.scalar.dma_start(out=yv[:, t0 : t0 + g, :], in_=og[:, :, :])
            c += rpc * s
        assert c == per_core_total
    nc.finalize()
    return nc


def _keep_mask_u8(total):
    """Reproduce the reference dropout keep mask bit-exactly (jax threefry)."""
    import jax

    try:
        dev = jax.devices("cpu")[0]
    except RuntimeError:
        dev = None
    if dev is not None:
        with jax.default_device(dev):
            k = jax.random.bernoulli(
                jax.random.key(42), 1.0 - _DROPOUT_P, (total,)
            )
            k = np.asarray(k)
    else:
        k = np.asarray(
            jax.random.bernoulli(jax.random.key(42), 1.0 - _DROPOUT_P, (total,))
        )
    return k.astype(np.uint8)


def kernel(input, mask, seqlen, batch, heads):
    from concourse.bass_utils import run_bass_kernel_spmd

    global LAST_RESULTS

    x = np.ascontiguousarray(np.asarray(input, dtype=np.float32).reshape(-1))
    m = np.ascontiguousarray(np.asarray(mask, dtype=np.float32).reshape(-1))
    seqs = [int(v) for v in np.asarray(seqlen).reshape(-1)]
    b = int(batch)
    h = int(heads)
    seqs = seqs[:b]
    assert all(h * s % _N_CORES == 0 for s in seqs)
    assert all((h * s // _N_CORES) % 128 == 0 for s in seqs)

    total = int(sum(h * s * s for s in seqs))
    mask_total = int(sum(seqs))
    assert x.size == total and m.size == mask_total

    seg_shapes = [((h * s) // _N_CORES, s) for s in seqs]
    per_core_total = sum(r * s for r, s in seg_shapes)
    moffs = np.concatenate([[0], np.cumsum(seqs)])[:-1].astype(np.int64)
    bases = np.concatenate(
        [[0], np.cumsum([h * s * s for s in seqs])]
    )[:-1].astype(np.int64)

    # The fast path skips rowmax subtraction.  That is exact enough when
    # exp(x) cannot overflow f32 and no softmax row is entirely masked
    # (an all-masked row needs the max-subtracted degenerate 1/s output).
    finite_mask = m > -1e8
    all_masked_row = any(
        not finite_mask[mo : mo + s].any() for mo, s in zip(moffs, seqs)
    )
    use_max = bool(all_masked_row or (x.size and np.abs(x).max() > 60.0))

    key = (tuple(seqs), h, use_max)
    if key not in _PROGRAM_CACHE:
        _PROGRAM_CACHE[key] = _build_bass(
            seg_shapes, list(moffs), mask_total, per_core_total, use_max=use_max
        )
    nc = _PROGRAM_CACHE[key]

    keep = _keep_mask_u8(total)

    # shard + permute: core d gets, for each batch i, the contiguous flat
    # span [base_i + d*n, base_i + (d+1)*n) (n = rpc_i*s_i), reordered from
    # [T,128,s] row-major tiles to partition-major [128,T,s].
    in_maps = []
    for d in range(_N_CORES):
        xs = np.empty(per_core_total, dtype=np.float32)
        ks = np.empty(per_core_total, dtype=np.uint8)
        c = 0
        for i, (r, s) in enumerate(seg_shapes):
            n = r * s
            T = r // 128
            src = slice(bases[i] + d * n, bases[i] + (d + 1) * n)
            xs[c : c + n] = (
                x[src].reshape(T, 128, s).transpose(1, 0, 2).reshape(-1)
            )
            ks[c : c + n] = (
                keep[src].reshape(T, 128, s).transpose(1, 0, 2).reshape(-1)
            )
            c += n
        in_maps.append({"x": xs, "keep": ks, "mask": m})

    trace = bool(os.environ.get("KERNEL_TRACE"))
    if not trace:
        # an ambient BASS_TRACE would route through the axon NTFF hook,
        # which this image only has when the test harness installs it
        os.environ["BASS_NEVER_TRACE"] = "1"
    res = run_bass_kernel_spmd(
        nc, in_maps, core_ids=list(range(_N_CORES)), trace=trace
    )
    LAST_RESULTS = res

    out = np.empty(total, dtype=np.float32)
    for d in range(_N_CORES):
        yd = res.results[d]["y"]
        c = 0
        for i, (r, s) in enumerate(seg_shapes):
            n = r * s
            T = r // 128
            out[bases[i] + d * n : bases[i] + (d + 1) * n] = (
                yd[c : c + n].reshape(128, T, s).transpose(1, 0, 2).reshape(-1)
            )
            c += n
    return out


# revision 22
# speedup vs baseline: 1.2454x; 1.1750x over previous
"""Ragged masked softmax + dropout kernel for 8 Trainium2 NeuronCores.

Computation (per softmax row of length s_i, batch i, any head):
    x' = x + mask_i            (mask_i broadcasts over all heads*s_i rows)
    out = exp(x' - max(x')) / sum(exp(x' - max(x'))) * keep / (1 - p)
where keep = jax.random.bernoulli(key(42), 1-p, (total,)) -- reproduced
bit-exactly on host (jax CPU threefry) and shipped as uint8.

Sharding: each batch i has heads*s_i rows of length s_i, contiguous in the
flat buffer.  Core d owns rows [d*rpc_i, (d+1)*rpc_i) of EVERY batch
(rpc_i = heads*s_i/8), i.e. one contiguous flat span per batch per core.
All cores get identical-size, identical-structure shards -> one SPMD program.

Device layout: per-core shards are host-permuted per segment from
[T*128 rows, s] row-major to partition-major [128][T][s] so every DMA moves
long contiguous per-partition lines (k*s*4B for x/y groups, T*s B for the
whole keep segment) instead of 1-4KB row lines.  Loads issue on the sync
HWDGE ring, stores on the scalar (ACT) ring.
"""

import os
import sys

import numpy as np

os.environ.setdefault("MYCRO_LOCAL_CACHE", "1")

for _p in ("/opt/trn_rl_repo", "/root/.axon_site/_ro/trn_rl_repo"):
    if os.path.isdir(_p) and _p not in sys.path:
        sys.path.append(_p)

_N_CORES = 8
_DROPOUT_P = 0.1

_PROGRAM_CACHE = {}
# exposed for test harness: BassKernelResults of the most recent run
LAST_RESULTS = None


def _groups(T, s, split_first=False):
    """Split T tiles into groups with ~16KB f32 per-partition DMA lines.
    split_first carves a 1-tile group off the front so the first compute
    can start as soon as one tile's worth of data lands."""
    k = max(1, min(T, 16384 // (4 * s)))
    out = []
    t = 0
    if split_first and k > 1:
        out.append((0, 1))
        t = 1
    while t < T:
        g = min(k, T - t)
        out.append((t, g))
        t += g
    return out


def _build_bass(seg_shapes, mask_offsets, mask_total, per_core_total,
                use_max=False):
    """seg_shapes: list of (rows_per_core, seqlen); mask_offsets: per-segment
    offsets into the flat mask tensor.  x/keep/y DRAM tensors hold the
    host-permuted partition-major layout described in the module docstring.

    use_max=False skips the rowmax subtraction (safe for bounded inputs:
    exp(x+m) cannot overflow and masked entries underflow to exactly 0);
    use_max=True is the numerically-safe general path."""
    import concourse.bass as bass
    import concourse.bacc as bacc
    import concourse.tile as tile
    from concourse import mybir
    from contextlib import ExitStack

    f32 = mybir.dt.float32
    u8 = mybir.dt.uint8
    EXP = mybir.ActivationFunctionType.Exp
    COPY = mybir.ActivationFunctionType.Copy
    ADD = mybir.AluOpType.add
    MAX = mybir.AluOpType.max
    MULT = mybir.AluOpType.mult
    inv_keep = float(1.0 / (1.0 - _DROPOUT_P))

    nc = bacc.Bacc("TRN2", target_bir_lowering=False)
    x = nc.dram_tensor("x", [per_core_total], f32, kind="ExternalInput")
    keep = nc.dram_tensor("keep", [per_core_total], u8, kind="ExternalInput")
    mask = nc.dram_tensor("mask", [mask_total], f32, kind="ExternalInput")
    y = nc.dram_tensor("y", [per_core_total], f32, kind="ExternalOutput")

    with tile.TileContext(nc) as tc, ExitStack() as ctx:
        xp = ctx.enter_context(tc.tile_pool(name="xp", bufs=4))
        kp = ctx.enter_context(tc.tile_pool(name="kp", bufs=2))
        xmp = ctx.enter_context(tc.tile_pool(name="xmp", bufs=3))
        ep = ctx.enter_context(tc.tile_pool(name="ep", bufs=3))
        op = ctx.enter_context(tc.tile_pool(name="op", bufs=3))
        mp = ctx.enter_context(tc.tile_pool(name="mp", bufs=2))
        st = ctx.enter_context(tc.tile_pool(name="st", bufs=8))

        c = 0
        for (rpc, s), moff in zip(seg_shapes, mask_offsets):
            T = (rpc + 127) // 128
            assert rpc == T * 128, "permuted layout needs rpc % 128 == 0"

            mt = mp.tile([128, s], f32, tag="mt")
            msrc = mask[moff : moff + s]
            bsrc = bass.AP(
                tensor=msrc.tensor,
                offset=msrc.offset,
                ap=[[0, 128]] + [list(a) for a in msrc.ap],
            )
            nc.gpsimd.dma_start(out=mt[:, :], in_=bsrc)

            # permuted views: [128 partitions, T, s]
            xv = x[c : c + rpc * s].rearrange("(p t s) -> p t s", p=128, s=s)
            kv = keep[c : c + rpc * s].rearrange("(p t s) -> p t s", p=128, s=s)
            yv = y[c : c + rpc * s].rearrange("(p t s) -> p t s", p=128, s=s)

            # whole keep segment in one DMA (contiguous T*s per partition);
            # issued after the first x group so first compute isn't stuck
            # behind it in the sync-ring FIFO
            kt = kp.tile([128, T, s], u8, tag="kt")
            kt_issued = False

            for t0, g in _groups(T, s, split_first=(c == 0)):
                xg = xp.tile([128, g, s], f32, tag="xg")
                nc.sync.dma_start(out=xg[:, :, :], in_=xv[:, t0 : t0 + g, :])
                if not kt_issued:
                    nc.sync.dma_start(out=kt[:, :, :], in_=kv)
                    kt_issued = True
                og = op.tile([128, g, s], f32, tag="og")

                for j in range(g):
                    # xm = x + mask
                    xm = xmp.tile([128, s], f32, tag="xm")
                    nc.vector.tensor_tensor(xm[:], xg[:, j, :], mt[:], ADD)
                    if use_max:
                        mn = st.tile([128, 1], f32, tag="mn")
                        nc.vector.tensor_reduce(
                            mn[:],
                            xm[:],
                            axis=mybir.AxisListType.X,
                            op=MAX,
                            negate=True,
                        )
                    # e = exp(xm [- rowmax]); sm = rowsum(e)
                    et = ep.tile([128, s], f32, tag="et")
                    sm = st.tile([128, 1], f32, tag="sm")
                    nc.scalar.activation(
                        out=et[:],
                        in_=xm[:],
                        func=EXP,
                        bias=mn[:] if use_max else 0.0,
                        scale=1.0,
                        accum_out=sm[:],
                    )
                    rc = st.tile([128, 1], f32, tag="rc")
                    nc.vector.reciprocal(out=rc[:], in_=sm[:])
                    rcs = st.tile([128, 1], f32, tag="rcs")
                    nc.vector.tensor_scalar_mul(rcs[:], rc[:], inv_keep)
                    # out = (e * 1/(0.9*s)) * keep in one fused DVE op
                    # (u8 keep is converted on the fly)
                    nc.vector.scalar_tensor_tensor(
                        og[:, j, :],
                        et[:],
                        rcs[:],
                        kt[:, t0 + j, :],
                        op0=MULT,
                        op1=MULT,
                    )
                # store the whole group on the ACT HWDGE ring
                nc.scalar.dma_start(out=yv[:, t0 : t0 + g, :], in_=og[:, :, :])
            c += rpc * s
        assert c == per_core_total
    nc.finalize()
    return nc


def _keep_mask_u8(total):
    """Reproduce the reference dropout keep mask bit-exactly (jax threefry)."""
    import jax

    try:
        dev = jax.devices("cpu")[0]
    except RuntimeError:
        dev = None
    if dev is not None:
        with jax.default_device(dev):
            k = jax.random.bernoulli(
                jax.random.key(42), 1.0 - _DROPOUT_P, (total,)
            )
            k = np.asarray(k)
    else:
        k = np.asarray(
            jax.random.bernoulli(jax.random.key(42), 1.0 - _DROPOUT_P, (total,))
        )
    return k.astype(np.uint8)


def kernel(input, mask, seqlen, batch, heads):
    from concourse.bass_utils import run_bass_kernel_spmd

    global LAST_RESULTS

    x = np.ascontiguousarray(np.asarray(input, dtype=np.float32).reshape(-1))
    m = np.ascontiguousarray(np.asarray(mask, dtype=np.float32).reshape(-1))
    seqs = [int(v) for v in np.asarray(seqlen).reshape(-1)]
    b = int(batch)
    h = int(heads)
    seqs = seqs[:b]
    assert all(h * s % _N_CORES == 0 for s in seqs)
    assert all((h * s // _N_CORES) % 128 == 0 for s in seqs)

    total = int(sum(h * s * s for s in seqs))
    mask_total = int(sum(seqs))
    assert x.size == total and m.size == mask_total

    seg_shapes = [((h * s) // _N_CORES, s) for s in seqs]
    per_core_total = sum(r * s for r, s in seg_shapes)
    moffs = np.concatenate([[0], np.cumsum(seqs)])[:-1].astype(np.int64)
    bases = np.concatenate(
        [[0], np.cumsum([h * s * s for s in seqs])]
    )[:-1].astype(np.int64)

    # The fast path skips rowmax subtraction.  That is exact enough when
    # exp(x) cannot overflow f32 and no softmax row is entirely masked
    # (an all-masked row needs the max-subtracted degenerate 1/s output).
    finite_mask = m > -1e8
    all_masked_row = any(
        not finite_mask[mo : mo + s].any() for mo, s in zip(moffs, seqs)
    )
    use_max = bool(all_masked_row or (x.size and np.abs(x).max() > 60.0))

    key = (tuple(seqs), h, use_max)
    if key not in _PROGRAM_CACHE:
        _PROGRAM_CACHE[key] = _build_bass(
            seg_shapes, list(moffs), mask_total, per_core_total, use_max=use_max
        )
    nc = _PROGRAM_CACHE[key]

    keep = _keep_mask_u8(total)

    # shard + permute: core d gets, for each batch i, the contiguous flat
    # span [base_i + d*n, base_i + (d+1)*n) (n = rpc_i*s_i), reordered from
    # [T,128,s] row-major tiles to partition-major [128,T,s].
    in_maps = []
    for d in range(_N_CORES):
        xs = np.empty(per_core_total, dtype=np.float32)
        ks = np.empty(per_core_total, dtype=np.uint8)
        c = 0
        for i, (r, s) in enumerate(seg_shapes):
            n = r * s
            T = r // 128
            src = slice(bases[i] + d * n, bases[i] + (d + 1) * n)
            xs[c : c + n] = (
                x[src].reshape(T, 128, s).transpose(1, 0, 2).reshape(-1)
            )
            ks[c : c + n] = (
                keep[src].reshape(T, 128, s).transpose(1, 0, 2).reshape(-1)
            )
            c += n
        in_maps.append({"x": xs, "keep": ks, "mask": m})

    trace = bool(os.environ.get("KERNEL_TRACE"))
    if not trace:
        # an ambient BASS_TRACE would route through the axon NTFF hook,
        # which this image only has when the test harness installs it
        os.environ["BASS_NEVER_TRACE"] = "1"
    res = run_bass_kernel_spmd(
        nc, in_maps, core_ids=list(range(_N_CORES)), trace=trace
    )
    LAST_RESULTS = res

    out = np.empty(total, dtype=np.float32)
    for d in range(_N_CORES):
        yd = res.results[d]["y"]
        c = 0
        for i, (r, s) in enumerate(seg_shapes):
            n = r * s
            T = r // 128
            out[bases[i] + d * n : bases[i] + (d + 1) * n] = (
                yd[c : c + n].reshape(128, T, s).transpose(1, 0, 2).reshape(-1)
            )
            c += n
    return out
